# revision 14
# baseline (speedup 1.0000x reference)
"""Trainium2 Bass kernel for MultiHeadedRelAttention.

Reference computation (B=4, L=256, D=256, H=8, DH=32):
    q = shape(x @ Wq.T + bq) / sqrt(DH)
    k = shape(x @ Wk.T + bk); v = shape(x @ Wv.T + bv)
    a_c  = einsum('bhqd,bhkd->bhqk', q + u, k)
    b_d  = einsum('bhqd,bqkhd->bhqk', q + v, rel_emb.reshape(B,L,L,H,DH))
    attn = softmax(where(mask, -1e18, a_c + b_d))
    out  = (attn @ v).merge_heads() @ Wo.T + bo
    returns (out, attn)

Sharding: 8 cores; core c handles batch b=c//2, query rows [q0, q0+128),
q0=(c%2)*128.  rel_emb (256MB) dominates traffic -> 32MB per core.

Per-core device layout:
  - Scores live in PSUM as one (128, 8, 256) tile: partition p = qi16*8 + h/2*..
    precisely p = (qi % 16)*8 + h for group g = qi//16, free = (g, k).
    Every partition row is an independent (qi, h) softmax row.
  - b_d: per qi, 2 accumulating matmuls with K=128: lhsT is a block-masked
    "Qblk" (d, 4 heads) built from qv^T, rhs = rel^T[qi] d-chunk (128, 256 k).
  - a_c: 2 more accumulating matmuls per qi with Qblk built from qu^T and
    rhs = k^T d-chunk.
  - softmax per group on (128, 256) rows (full lane utilization).
  - ctx via PE-transposed attn tiles, output projection on PE.
"""

import os
import sys

import numpy as np

for _p in ("/opt/trn_rl_repo", "/root/.axon_site/_ro/trn_rl_repo"):
    if os.path.isdir(_p) and _p not in sys.path:
        sys.path.insert(0, _p)

B, L, D, H = 4, 256, 256, 8
DH = D // H  # 32
P = 128  # SBUF partitions
QC = 128  # query rows per core
NG = QC // 16  # 8 groups of 16 qi
N_CORES = 8
INV_SQRT_DH = 1.0 / float(np.sqrt(DH))
MASK_NEG = -1.0e18

_PROGRAM_CACHE = {}


def build_program():
    """Build the single-core Bass program (SPMD: same program on all cores)."""
    from contextlib import ExitStack

    import concourse.bass as bass
    import concourse.mybir as mybir
    import concourse.tile as tile
    from concourse import bacc
    from concourse.masks import make_identity

    f32 = mybir.dt.float32
    nc = bacc.Bacc(None, target_bir_lowering=False, debug=False)

    # ---- I/O ----
    xT = nc.declare_dram_parameter("xT", [D, L], f32, isOutput=False)
    xqT = nc.declare_dram_parameter("xqT", [D, QC], f32, isOutput=False)
    wqT = nc.declare_dram_parameter("wqT", [D, D], f32, isOutput=False)
    wkT = nc.declare_dram_parameter("wkT", [D, D], f32, isOutput=False)
    wvT = nc.declare_dram_parameter("wvT", [D, D], f32, isOutput=False)
    woT = nc.declare_dram_parameter("woT", [D, D], f32, isOutput=False)
    bqu = nc.declare_dram_parameter("bqu", [D], f32, isOutput=False)
    bqv = nc.declare_dram_parameter("bqv", [D], f32, isOutput=False)
    bk = nc.declare_dram_parameter("bk", [D], f32, isOutput=False)
    bv = nc.declare_dram_parameter("bv", [D], f32, isOutput=False)
    bo = nc.declare_dram_parameter("bo", [D], f32, isOutput=False)
    maskb = nc.declare_dram_parameter("maskb", [QC, L], f32, isOutput=False)
    relT = nc.declare_dram_parameter("relT", [QC, D, L], f32, isOutput=False)

    out_p = nc.declare_dram_parameter("out_p", [QC, D], f32, isOutput=True)
    attn_p = nc.declare_dram_parameter("attn_p", [H, QC, L], f32, isOutput=True)

    Alu = mybir.AluOpType
    Act = mybir.ActivationFunctionType

    with ExitStack() as ctx:
        tc = ctx.enter_context(tile.TileContext(nc))
        consts = ctx.enter_context(tc.tile_pool(name="consts", bufs=1))
        qkv = ctx.enter_context(tc.tile_pool(name="qkv", bufs=1))
        stats = ctx.enter_context(tc.tile_pool(name="stats", bufs=16))

        # ---- constants / inputs resident in SBUF ----
        xT_sb = [consts.tile([P, L], f32, tag=f"xT{c}", name=f"xT{c}") for c in range(2)]
        xqT_sb = [consts.tile([P, QC], f32, tag=f"xqT{c}", name=f"xqT{c}") for c in range(2)]
        wq_sb = [consts.tile([P, D], f32, tag=f"wq{c}", name=f"wq{c}") for c in range(2)]
        wk_sb = [consts.tile([P, D], f32, tag=f"wk{c}", name=f"wk{c}") for c in range(2)]
        wv_sb = [consts.tile([P, D], f32, tag=f"wv{c}", name=f"wv{c}") for c in range(2)]
        wo_sb = [consts.tile([P, D], f32, tag=f"wo{c}", name=f"wo{c}") for c in range(2)]
        for c in range(2):
            sl = slice(c * P, (c + 1) * P)
            nc.sync.dma_start(out=xT_sb[c][:], in_=xT[sl, :])
            nc.sync.dma_start(out=xqT_sb[c][:], in_=xqT[sl, :])
            nc.sync.dma_start(out=wq_sb[c][:], in_=wqT[sl, :])
            nc.sync.dma_start(out=wk_sb[c][:], in_=wkT[sl, :])
            nc.sync.dma_start(out=wv_sb[c][:], in_=wvT[sl, :])
            nc.sync.dma_start(out=wo_sb[c][:], in_=woT[sl, :])

        # bias columns: tile (128, 2), column c = vec[c*128:(c+1)*128]
        def bias_col(vec, name):
            t = consts.tile([P, 2], f32, tag=name, name=name)
            nc.sync.dma_start(out=t[:], in_=vec.rearrange("(c p) -> p c", p=P))
            return t

        bqu_sb = bias_col(bqu, "bqu")
        bqv_sb = bias_col(bqv, "bqv")
        bk_sb = bias_col(bk, "bk")
        # bv, bo broadcast along partitions (bias lives on the free dim there)
        def row_bcast(vec):
            a = vec[:]
            return bass.AP(tensor=a.tensor, offset=a.offset, ap=[[0, P], [1, D]])

        bv_sb = consts.tile([P, D], f32, tag="bv")
        nc.sync.dma_start(out=bv_sb[:], in_=row_bcast(bv))
        bo_sb = consts.tile([P, D], f32, tag="bo")
        nc.sync.dma_start(out=bo_sb[:], in_=row_bcast(bo))

        # interleaved additive mask: partition p = qi4*32 + r (the mask row of
        # qi = g*4 + qi4, replicated over the 32 rows of its block), free=(g,k)
        mask_sb = consts.tile([P, QC // 4, L], f32, tag="mask")
        _m = maskb[:]
        for g in range(QC // 4):
            mask_src = bass.AP(
                tensor=_m.tensor,
                offset=_m.offset + g * 4 * L,
                ap=[[L, 4], [0, 32], [1, L]],
            )
            nc.sync.dma_start(out=mask_sb[:, g, :], in_=mask_src)

        identity = consts.tile([P, P], f32, tag="ident")
        make_identity(nc, identity[:])

        # 0/1 head-segment masks: (128, 4), col hl = rows [32*hl, 32*hl+32)
        seg = consts.tile([P, 4], f32, tag="seg")
        nc.vector.memset(seg[:], 0.0)
        for hl in range(4):
            nc.vector.memset(seg[hl * DH : (hl + 1) * DH, hl : hl + 1], 1.0)

        # ---- stage B: projections (q^T, k^T, v natural) ----
        quT = [qkv.tile([P, QC], f32, tag=f"quT{c}", name=f"quT{c}") for c in range(2)]
        qvT = [qkv.tile([P, QC], f32, tag=f"qvT{c}", name=f"qvT{c}") for c in range(2)]
        kT = [qkv.tile([P, L], f32, tag=f"kT{c}", name=f"kT{c}") for c in range(2)]
        v_sb = [qkv.tile([P, D], f32, tag=f"v{c}", name=f"v{c}") for c in range(2)]

        pmm = ctx.enter_context(tc.tile_pool(name="pmm", bufs=1, space="PSUM"))
        if True:
            for c in range(2):  # output d chunk
                osl = slice(c * P, (c + 1) * P)
                psq = pmm.tile([P, QC], f32, tag="pmm")
                for ci in range(2):  # contraction chunk
                    nc.tensor.matmul(
                        psq[:],
                        wq_sb[ci][:, osl],
                        xqT_sb[ci][:],
                        start=(ci == 0),
                        stop=(ci == 1),
                    )
                # quT = psq/sqrt(DH) + (bq/sqrt(DH)+u);  qvT likewise with v
                nc.vector.tensor_scalar(
                    out=quT[c][:], in0=psq[:],
                    scalar1=INV_SQRT_DH, scalar2=bqu_sb[:, c : c + 1],
                    op0=Alu.mult, op1=Alu.add,
                )
                nc.vector.tensor_scalar(
                    out=qvT[c][:], in0=psq[:],
                    scalar1=INV_SQRT_DH, scalar2=bqv_sb[:, c : c + 1],
                    op0=Alu.mult, op1=Alu.add,
                )

                psk = pmm.tile([P, L], f32, tag="pmm")
                for ci in range(2):
                    nc.tensor.matmul(
                        psk[:],
                        wk_sb[ci][:, osl],
                        xT_sb[ci][:],
                        start=(ci == 0),
                        stop=(ci == 1),
                    )
                nc.vector.tensor_scalar(
                    out=kT[c][:], in0=psk[:],
                    scalar1=bk_sb[:, c : c + 1], scalar2=None, op0=Alu.add,
                )

                # v natural: out rows = sequence positions chunk c
                psv = pmm.tile([P, D], f32, tag="pmm")
                for ci in range(2):
                    nc.tensor.matmul(
                        psv[:],
                        xT_sb[ci][:, osl],
                        wv_sb[ci][:],
                        start=(ci == 0),
                        stop=(ci == 1),
                    )
                nc.vector.tensor_add(v_sb[c][:], psv[:], bv_sb[:])

        # ---- stage B2: block-masked Qblk lhsT tiles ----
        # Score-group layout: 32 groups of 4 qi; PSUM tile (128, 256) per
        # group with partition p = (qi%4)*32 + h (rows 8..32 of each block
        # are zeroed by the start matmul and ignored).
        #
        # Per qi, 4 accumulating matmuls at base (qi%4)*32:
        #   rel chunk0: lhsT = qpad_v0[:, qi, :]  (M=32: 4 head cols + 28 zero,
        #               start=True -> zeroes the whole 32-row block)
        #   rel chunk1: lhsT = qblk_v1[:, qi, :]  (M=8: 4 zero + heads 4..7)
        #   k   chunk0: lhsT = qblk_u0[:, qi, :]  (M=4: heads 0..3)
        #   k   chunk1: lhsT = qblk_u1[:, qi, :]  (M=8: 4 zero + heads 4..7)
        qpad_v0 = qkv.tile([P, QC, 32], f32, tag="qpad_v0")
        qblk_v1 = qkv.tile([P, QC, 8], f32, tag="qblk_v1")
        qblk_u0 = qkv.tile([P, QC, 4], f32, tag="qblk_u0")
        qblk_u1 = qkv.tile([P, QC, 8], f32, tag="qblk_u1")
        nc.vector.memset(qpad_v0[:], 0.0)
        nc.vector.memset(qblk_v1[:], 0.0)
        nc.vector.memset(qblk_u1[:], 0.0)
        for hl in range(4):
            nc.vector.tensor_scalar(
                out=qpad_v0[:, :, hl], in0=qvT[0][:],
                scalar1=seg[:, hl : hl + 1], scalar2=None, op0=Alu.mult,
            )
            nc.vector.tensor_scalar(
                out=qblk_v1[:, :, 4 + hl], in0=qvT[1][:],
                scalar1=seg[:, hl : hl + 1], scalar2=None, op0=Alu.mult,
            )
            nc.vector.tensor_scalar(
                out=qblk_u0[:, :, hl], in0=quT[0][:],
                scalar1=seg[:, hl : hl + 1], scalar2=None, op0=Alu.mult,
            )
            nc.vector.tensor_scalar(
                out=qblk_u1[:, :, 4 + hl], in0=quT[1][:],
                scalar1=seg[:, hl : hl + 1], scalar2=None, op0=Alu.mult,
            )

        # ---- stage C: rel stream -> scores -> softmax ----
        NGR = QC // 4  # 32 score groups of 4 qi
        relp = ctx.enter_context(tc.tile_pool(name="relp", bufs=8))
        psc = ctx.enter_context(tc.tile_pool(name="psc", bufs=3, space="PSUM"))
        attnp = ctx.enter_context(tc.tile_pool(name="attnp", bufs=4))
        attnT = qkv.tile([P, 2, NGR, P], f32, tag="attnT")
        ptr = ctx.enter_context(tc.tile_pool(name="ptr", bufs=1, space="PSUM"))

        for g in range(NGR):
            scores = psc.tile([P, L], f32, tag="scores")
            for j in range(4):
                qi = g * 4 + j
                base = j * 32
                rel_t = relp.tile([P, 2, L], f32, tag="rel")
                nc.sync.dma_start(
                    out=rel_t[:], in_=relT[qi].rearrange("(c p) k -> p c k", p=P)
                )
                tp = (0, base)
                nc.tensor.matmul(
                    scores[base : base + 32, :], qpad_v0[:, qi, :], rel_t[:, 0, :],
                    start=True, stop=False, skip_group_check=True,
                    tile_position=tp,
                )
                nc.tensor.matmul(
                    scores[base : base + 8, :], qblk_v1[:, qi, :], rel_t[:, 1, :],
                    start=False, stop=False, skip_group_check=True,
                    tile_position=tp,
                )
                nc.tensor.matmul(
                    scores[base : base + 4, :], qblk_u0[:, qi, :], kT[0][:],
                    start=False, stop=False, skip_group_check=True,
                    tile_position=tp,
                )
                nc.tensor.matmul(
                    scores[base : base + 8, :], qblk_u1[:, qi, :], kT[1][:],
                    start=False, stop=True, skip_group_check=True,
                    tile_position=tp,
                )

            # softmax over k for the 32 valid (qi, h) rows (zero rows harmless)
            attn_t = attnp.tile([P, L], f32, tag="attn")
            sg = attn_t[:]
            nc.vector.tensor_tensor(sg, scores[:], mask_sb[:, g, :], Alu.add)
            nmax = stats.tile([P, 1], f32, tag="nmax")
            nc.vector.reduce_max(out=nmax[:], in_=sg, axis=mybir.AxisListType.X,
                                 negate=True)
            ssum = stats.tile([P, 1], f32, tag="ssum")
            nc.scalar.activation(out=sg, in_=sg, func=Act.Exp,
                                 bias=nmax[:, 0:1], scale=1.0, accum_out=ssum[:])
            rsum = stats.tile([P, 1], f32, tag="rsum")
            nc.vector.reciprocal(out=rsum[:], in_=ssum[:])
            nc.vector.tensor_scalar(out=sg, in0=sg, scalar1=rsum[:, 0:1],
                                    scalar2=None, op0=Alu.mult)

            # attn out: rows a*32..a*32+8 hold heads 0..8 of qi = g*4+a
            for a in range(4):
                nc.sync.dma_start(
                    out=attn_p[:, g * 4 + a, :],
                    in_=attn_t[a * 32 : a * 32 + H, :],
                )
            # transpose attn (both k chunks) for the ctx matmul
            for kc in range(2):
                ps_t = ptr.tile([P, P], f32, tag="ptr")
                nc.tensor.transpose(
                    ps_t[:], attn_t[:, kc * P : (kc + 1) * P], identity[:]
                )
                nc.vector.tensor_copy(attnT[:, kc, g, :], ps_t[:])

        # ---- stage D: ctx^T[d, qi] = sum_k v[k, d] attn^T[k, qi] ----
        # attnT free cols for head h are qi4*32 + h; slicing [g, h::32] over
        # all groups streams qi = 0..127 in order.
        pctx = ctx.enter_context(tc.tile_pool(name="pctx", bufs=2, space="PSUM"))
        # full-bank pitch (512 f32) so PSUM zero-region accounting stays
        # partition-aligned; only cols 0:128 are used
        ctxT_ps = [pctx.tile([P, 512], f32, tag="ctxT", name=f"ctxT{i}")
                   for i in range(2)]
        for h in range(H):
            rows = slice((h % 4) * DH, (h % 4) * DH + DH)
            for c in range(2):  # k contraction chunk
                nc.tensor.matmul(
                    ctxT_ps[h // 4][rows, 0:P],
                    v_sb[c][:, h * DH : (h + 1) * DH],
                    attnT[:, c, :, h::32],
                    start=(c == 0),
                    stop=(c == 1),
                    tile_position=(0, (h % 4) * DH),
                )
        ctxT_sb = qkv.tile([P, 2, P], f32, tag="ctxT_sb")
        for i in range(2):
            nc.vector.tensor_copy(ctxT_sb[:, i, :], ctxT_ps[i][:, 0:P])

        # ---- stage E: out = ctx @ Wo.T + bo ----
        pout = ctx.enter_context(tc.tile_pool(name="pout", bufs=1, space="PSUM"))
        out_ps = pout.tile([P, D], f32, tag="out_ps")
        for c in range(2):
            nc.tensor.matmul(
                out_ps[:], ctxT_sb[:, c, :], wo_sb[c][:],
                start=(c == 0), stop=(c == 1),
            )
        out_sb = qkv.tile([P, D], f32, tag="out_sb")
        nc.vector.tensor_add(out_sb[:], out_ps[:], bo_sb[:])
        nc.sync.dma_start(out=out_p[:], in_=out_sb[:])

    nc.compile()
    return nc


def make_core_inputs(inputs, mask, rel_emb, Wq, bq, Wk, bk, Wv, bv, Wo, bo,
                     u, v_param):
    """Host-side shard prep: per-core input dicts."""
    f32 = np.float32
    wqT = np.ascontiguousarray(np.asarray(Wq, f32).T)
    wkT = np.ascontiguousarray(np.asarray(Wk, f32).T)
    wvT = np.ascontiguousarray(np.asarray(Wv, f32).T)
    woT = np.ascontiguousarray(np.asarray(Wo, f32).T)
    bq = np.asarray(bq, f32)
    bqu = (bq * INV_SQRT_DH + np.asarray(u, f32)).astype(f32)
    bqv = (bq * INV_SQRT_DH + np.asarray(v_param, f32)).astype(f32)
    bk = np.asarray(bk, f32)
    bv = np.asarray(bv, f32)
    bo = np.asarray(bo, f32)
    inputs = np.asarray(inputs, f32)
    rel_emb = np.asarray(rel_emb, f32)
    mask = np.asarray(mask)

    in_maps = []
    for core in range(N_CORES):
        b = core // 2
        q0 = (core % 2) * QC
        xb = inputs[b]  # (L, D)
        in_maps.append({
            "xT": np.ascontiguousarray(xb.T),
            "xqT": np.ascontiguousarray(xb[q0 : q0 + QC].T),
            "wqT": wqT, "wkT": wkT, "wvT": wvT, "woT": woT,
            "bqu": bqu, "bqv": bqv, "bk": bk, "bv": bv, "bo": bo,
            "maskb": np.where(mask[b, q0 : q0 + QC], f32(MASK_NEG), f32(0.0)),
            "relT": np.ascontiguousarray(
                rel_emb[b, q0 : q0 + QC].transpose(0, 2, 1)
            ),
        })
    return in_maps


def assemble(results):
    """Gather per-core outputs into full (out, attn)."""
    out = np.empty((B, L, D), np.float32)
    attn = np.empty((B, H, L, L), np.float32)
    for core in range(N_CORES):
        b = core // 2
        q0 = (core % 2) * QC
        r = results[core]
        out[b, q0 : q0 + QC, :] = r["out_p"]
        attn[b, :, q0 : q0 + QC, :] = r["attn_p"]
    return out, attn


def kernel(inputs, mask, rel_emb, Wq, bq, Wk, bk, Wv, bv, Wo, bo, u, v_param):
    from concourse.bass_utils import run_bass_kernel_spmd

    in_maps = make_core_inputs(inputs, mask, rel_emb, Wq, bq, Wk, bk, Wv, bv,
                               Wo, bo, u, v_param)
    if "program" not in _PROGRAM_CACHE:
        _PROGRAM_CACHE["program"] = build_program()
    nc = _PROGRAM_CACHE["program"]
    res = run_bass_kernel_spmd(nc, in_maps, list(range(N_CORES)))
    return assemble(res.results)


if __name__ == "__main__":
    # smoke test with random data
    rng = np.random.default_rng(0)
    ins = {
        "inputs": rng.standard_normal((B, L, D), dtype=np.float32),
        "mask": rng.random((B, L, L)) < 0.15,
        "rel_emb": (rng.standard_normal((B, L, L, D), dtype=np.float32) * 0.05),
        "Wq": rng.standard_normal((D, D), dtype=np.float32) / 16,
        "bq": np.zeros(D, np.float32),
        "Wk": rng.standard_normal((D, D), dtype=np.float32) / 16,
        "bk": np.zeros(D, np.float32),
        "Wv": rng.standard_normal((D, D), dtype=np.float32) / 16,
        "bv": np.zeros(D, np.float32),
        "Wo": rng.standard_normal((D, D), dtype=np.float32) / 16,
        "bo": np.zeros(D, np.float32),
        "u": rng.standard_normal(D).astype(np.float32),
        "v_param": rng.standard_normal(D).astype(np.float32),
    }
    out, attn = kernel(**ins)
    print(out.shape, attn.shape)
